# revision 16
# baseline (speedup 1.0000x reference)
"""Chamfer loss kernel for Trainium2 (8 NeuronCores).

Problem: pred/target [4, 3, 8192] channel-first point clouds.
loss = mean_i min_j ||p_i - t_j|| + mean_j min_i ||p_i - t_j||

d2[i,j] = ||p_i||^2 + ||t_j||^2 - 2 p_i.t_j is expressed as a single
K=16 fp16 matmul per tile (hi/lo splits keep |err| ~1e-6).  sqrt is
monotonic, so mins are taken over d2 and sqrt'd on host.

Sharding: core c -> (batch b = c//2, pred-row half h = c%2).  Each core
computes a [4096, 8192] block of d2 as 32 row tiles x 4 chunks of
[128, 2048] in PSUM and extracts
  - row mins  (min over the 8192 cols)  -> rowmin [128, 32]
  - col mins  (partial, per-partition)  -> colacc [128, 8192] -> PE
    transpose + reduce -> colmin [128, 64]
Host combines the tiny outputs.

The post-matmul reduction is the bottleneck.  ScalarE exports each PSUM
tile to SBUF fp16 (1 elem/cycle/lane); VectorE does both min directions
on the fp16 data at its 2x packed rate: one wide tensor_tensor min per
row tile into colacc, and a 4-level pairwise fold tree + tensor_reduce
for the row mins.  fp32 accumulation and the PE-transpose finale give
the cross-partition column mins.

Notes from exploration (this toolchain, axon/walrus):
  - nc.gpsimd.tensor_tensor/"Pool TensorTensor" fails walrus codegen
    (ISA engine check) - GPSIMD cannot help with elementwise min.
  - nc.vector.tensor_tensor_reduce compiles but crashes the device
    (NRT_EXEC_UNIT_UNRECOVERABLE) in every variant tried.
  - nc.vector.pool_max runs at 1x (no packed mode) - slower than the
    2x fold tree.
  - DMA cannot read PSUM (dma_start asserts SBUF/DRAM source).
Hence all reduction work lands on DVE (~303 us/core modeled busy),
ACT ~263 us, PE ~116 us; modeled total 320 us, measured 290-325 us
(session noise is +/-15%).

Each row tile is assigned a "way"; only "E" is usable here:
  E: ACT export fp16; DVE colmin TT + rowmin fold tree
  G/F/V (GPSIMD offload) and P (PSUM-direct DVE) are kept for
  reference but fail or lose on this toolchain.
"""

import numpy as np

B = 4
D = 3
N = 8192
HALF = N // 2  # pred rows per core
NCORES = 8
K = 16  # augmented contraction dim
RT = HALF // 128  # 32 row tiles per core
GW = 2048  # cols per PSUM tile (4 banks; 2 tiles in flight)
MMW = 512  # cols per matmul (one PSUM bank)
NT = N // 128  # 64 transpose blocks in the colmin finale

# Per-row-tile strategy, len 32.  r=0 must be E (its exports init colacc).
# GPSIMD (G/F/V) and tensor_tensor_reduce are rejected by this toolchain's
# walrus/runtime, so the default is all-E with the DVE fold tree.
WAYS_DEFAULT = "E" * 32

# "tree": baseline exact path (ACT fp16 export; DVE colacc TT + fold tree).
# "exp": softmin path — DEAD on this toolchain: the colmin pair-merge needs
#        TT(psumA, psumB) and walrus enforces "only one non-scalar input may
#        read PSUM" (NCC_IBVF027), leaving no cheap colmin source.
MODE = "tree"
CEXP = 80.0  # softmin sharpness: arg = CEXP * (1 - d2 / mhat_row)
PROBE = 512  # columns probed for the per-row upper bound mhat
MGUARD = 1e-4  # floor for mhat (negative-d2 / overflow guard)

_CACHE = {}


def _build_nc(ways=WAYS_DEFAULT, loop_n=None, rowmode="scan"):
    """loop_n: wrap the body in a device-side For_i loop executed loop_n
    times - constant program size, used for timing (delta between two
    loop_n values isolates pure HW execution time)."""
    import concourse.bacc as bacc
    import concourse.tile as tile
    from concourse import mybir

    assert len(ways) == RT and ways[0] == "E" and all(c in "EGFPV" for c in ways)
    f16 = mybir.dt.float16
    f32 = mybir.dt.float32
    MIN = mybir.AluOpType.min
    X = mybir.AxisListType.X
    BIG = 3.0e38

    uses_b = any(c in "GFV" for c in ways)

    nc = bacc.Bacc(
        "TRN2", target_bir_lowering=False, debug=False, num_devices=NCORES
    )
    stat = nc.dram_tensor("stat", [K, HALF], f16, kind="ExternalInput").ap()
    mov = nc.dram_tensor("mov", [K, N], f16, kind="ExternalInput").ap()
    ident = nc.dram_tensor("ident", [128, 128], f32, kind="ExternalInput").ap()
    mend = nc.dram_tensor("mend", [128, 1], f32, kind="ExternalInput").ap()
    rowmin_o = nc.dram_tensor("rowmin", [128, RT], f32, kind="ExternalOutput").ap()
    colmin_o = nc.dram_tensor("colmin", [128, NT], f32, kind="ExternalOutput").ap()

    with tile.TileContext(nc) as tc:
        with (
            tc.tile_pool(name="persist", bufs=1) as persist,
            tc.tile_pool(name="psum", bufs=2, space="PSUM") as psum_pool,
            tc.tile_pool(name="ckt", bufs=3) as ckt_pool,
            tc.tile_pool(name="scr", bufs=2) as scr_pool,
            tc.tile_pool(name="rp", bufs=2) as rp_pool,
        ):
            stat_sb = persist.tile([K, HALF], f16)
            mov_sb = persist.tile([K, N], f16)
            ident_sb = persist.tile([128, 128], f32)
            mend_sb = persist.tile([128, 1], f32)
            colacc = persist.tile([128, N], f16)
            colaccB = persist.tile([128, N], f32)
            rowmins = persist.tile([128, RT], f32)
            colmins = persist.tile([128, NT], f32)
            nc.sync.dma_start(stat_sb[:], stat)
            nc.sync.dma_start(mov_sb[:], mov)
            nc.sync.dma_start(ident_sb[:], ident)
            nc.sync.dma_start(mend_sb[:], mend)

            import contextlib

            loop_cm = (
                tc.For_i(0, loop_n, 1)
                if loop_n is not None
                else contextlib.nullcontext()
            )
            with loop_cm:
                b_inited = False
                for r, way in enumerate(ways):
                    lhsT = stat_sb[:, r * 128 : (r + 1) * 128]

                    if way == "P":
                        # PSUM-direct: both reductions read PSUM, no export.
                        rp = rp_pool.tile([128, 4], f32)
                        for g in range(4):
                            pt = psum_pool.tile([128, GW], f32, tag="pt")
                            for s in range(GW // MMW):
                                c0 = g * GW + s * MMW
                                nc.tensor.matmul(
                                    pt[:, s * MMW : (s + 1) * MMW],
                                    lhsT,
                                    mov_sb[:, c0 : c0 + MMW],
                                    start=True,
                                    stop=True,
                                )
                            csl = colacc[:, g * GW : (g + 1) * GW]
                            nc.vector.tensor_tensor(csl, pt[:], csl, MIN)
                            scr = scr_pool.tile([128, N // 2], f16)
                            nc.vector.tensor_tensor_reduce(
                                scr[:, : GW // 2],
                                pt[:, : GW // 2],
                                pt[:, GW // 2 :],
                                1.0,
                                BIG,
                                MIN,
                                MIN,
                                rp[:, g : g + 1],
                            )
                        nc.vector.tensor_reduce(
                            rowmins[:, r : r + 1], rp[:], X, MIN
                        )
                        continue

                    # Exported tiles.  First exported tile of each
                    # accumulator writes it directly (free init).
                    init_b = False
                    if way == "E" and r == 0:
                        dst = colacc
                    elif way in "GFV" and not b_inited:
                        dst = colaccB
                        b_inited = True
                        init_b = True
                    else:
                        dst = ckt_pool.tile([128, N], f16, tag="ck16")

                    for g in range(4):
                        pt = psum_pool.tile([128, GW], f32, tag="pt")
                        for s in range(GW // MMW):
                            c0 = g * GW + s * MMW
                            nc.tensor.matmul(
                                pt[:, s * MMW : (s + 1) * MMW],
                                lhsT,
                                mov_sb[:, c0 : c0 + MMW],
                                start=True,
                                stop=True,
                            )
                        dsl = dst[:, g * GW : (g + 1) * GW]
                        if way == "V":
                            nc.vector.tensor_copy(dsl, pt[:])
                        else:
                            nc.scalar.copy(dsl, pt[:])

                    # colmin merge
                    if way == "E":
                        if r > 0:
                            nc.vector.tensor_tensor(colacc[:], dst[:], colacc[:], MIN)
                    elif not init_b:  # G/F/V
                        nc.gpsimd.tensor_tensor(colaccB[:], dst[:], colaccB[:], MIN)

                    # rowmin
                    if way == "F":
                        nc.gpsimd.tensor_reduce(
                            rowmins[:, r : r + 1], dst[:], X, MIN
                        )
                    elif rowmode == "ttr":
                        scr = scr_pool.tile([128, N // 2], f16)
                        nc.vector.tensor_tensor_reduce(
                            scr[:],
                            dst[:, : N // 2],
                            dst[:, N // 2 :],
                            1.0,
                            BIG,
                            MIN,
                            MIN,
                            rowmins[:, r : r + 1],
                        )
                    elif rowmode == "ttr_bc":
                        # qr.py-style: dummy broadcast out, real accum
                        scr = scr_pool.tile([128, 1], f16, tag="scrbc")
                        nc.vector.tensor_tensor_reduce(
                            scr[:].broadcast_to((128, N // 2)),
                            dst[:, : N // 2],
                            dst[:, N // 2 :],
                            1.0,
                            BIG,
                            MIN,
                            MIN,
                            rowmins[:, r : r + 1],
                        )
                    elif rowmode == "ttr_add":
                        scr = scr_pool.tile([128, 1], f16, tag="scrbc")
                        nc.vector.tensor_tensor_reduce(
                            scr[:].broadcast_to((128, N // 2)),
                            dst[:, : N // 2],
                            dst[:, N // 2 :],
                            1.0,
                            0.0,
                            MIN,
                            mybir.AluOpType.add,
                            rowmins[:, r : r + 1],
                        )
                    elif rowmode == "ttr_rp":
                        scr = scr_pool.tile([128, N // 2], f16)
                        rp = rp_pool.tile([128, 4], f32)
                        nc.vector.tensor_tensor_reduce(
                            scr[:],
                            dst[:, : N // 2],
                            dst[:, N // 2 :],
                            1.0,
                            BIG,
                            MIN,
                            MIN,
                            rp[:, 0:1],
                        )
                        nc.vector.tensor_reduce(
                            rowmins[:, r : r + 1], rp[:, 0:1], X, MIN
                        )
                    elif rowmode == "pooltest":
                        # timing probe only: row-MAX via pool (wrong values)
                        nc.vector.pool_max(rowmins[:, r : r + 1], dst[:])
                    elif rowmode == "tmr":
                        # single 2x DVE instruction: full-range mask, min-accum
                        # (DEAD: InstTensorMaskReduce crashes the device, like
                        # InstTensorTensorReduce.)
                        scr = scr_pool.tile([128, N], f16, tag="tmrscr")
                        nc.vector.tensor_mask_reduce(
                            scr[:],
                            dst[:],
                            0.0,
                            mend_sb[:],
                            1.0,
                            BIG,
                            MIN,
                            accum_out=rowmins[:, r : r + 1],
                        )
                    elif rowmode == "scan":
                        # one 1x DVE scan over both tile halves:
                        #   state = min(state, dstL[t], dstR[t])
                        # final element = rowmin; extracted by ACT (slack).
                        scr = scr_pool.tile([128, N // 2], f16, tag="scanscr")
                        nc.vector.tensor_tensor_scan(
                            scr[:],
                            dst[:, : N // 2],
                            dst[:, N // 2 :],
                            BIG,
                            MIN,
                            MIN,
                        )
                        nc.scalar.copy(
                            rowmins[:, r : r + 1], scr[:, N // 2 - 1 : N // 2]
                        )
                    else:  # tree
                        scr = scr_pool.tile([128, N // 2], f16)
                        nc.vector.tensor_tensor(
                            scr[:], dst[:, : N // 2], dst[:, N // 2 :], MIN
                        )
                        scr2 = scr_pool.tile([128, N // 4], f16, tag="scr2")
                        nc.vector.tensor_tensor(
                            scr2[:], scr[:, : N // 4], scr[:, N // 4 :], MIN
                        )
                        scr3 = scr_pool.tile([128, N // 8], f16, tag="scr3")
                        nc.vector.tensor_tensor(
                            scr3[:], scr2[:, : N // 8], scr2[:, N // 8 :], MIN
                        )
                        scr4 = scr_pool.tile([128, N // 16], f16, tag="scr4")
                        nc.vector.tensor_tensor(
                            scr4[:], scr3[:, : N // 16], scr3[:, N // 16 :], MIN
                        )
                        scr5 = scr_pool.tile([128, N // 32], f16, tag="scr5")
                        nc.vector.tensor_tensor(
                            scr5[:], scr4[:, : N // 32], scr4[:, N // 32 :], MIN
                        )
                        nc.vector.tensor_reduce(
                            rowmins[:, r : r + 1], scr5[:], X, MIN
                        )

                # Fold the fp16 accumulator into the fp32 one; chunked so it
                # pipelines with the finale transposes.  The finale
                # transposes read fp32 (PE transpose out dtype must match).
                # The no-B cast-copy runs on ACT, which has slack.
                for q in range(4):
                    sl = slice(q * GW, (q + 1) * GW)
                    if uses_b:
                        nc.vector.tensor_tensor(
                            colaccB[:, sl], colacc[:, sl], colaccB[:, sl], MIN
                        )
                    else:
                        nc.scalar.copy(colaccB[:, sl], colacc[:, sl])

                # --- colmin finale: cross-partition reduce of colaccB ---
                # PE transpose of 128x128 blocks, packed min-reduce
                # 4 blocks per PSUM tile on DVE.
                for j in range(NT // 4):
                    pf = psum_pool.tile([128, GW], f32, tag="pt")
                    for kk in range(4):
                        t = 4 * j + kk
                        nc.tensor.matmul(
                            pf[:, kk * 128 : (kk + 1) * 128],
                            colaccB[:, t * 128 : (t + 1) * 128],
                            ident_sb[:],
                            is_transpose=True,
                            start=True,
                            stop=True,
                        )
                    nc.vector.tensor_reduce(
                        colmins[:, 4 * j : 4 * j + 4],
                        pf[:, :512].rearrange("p (b f) -> p b f", b=4),
                        X,
                        MIN,
                    )
            nc.sync.dma_start(rowmin_o, rowmins[:])
            nc.sync.dma_start(colmin_o, colmins[:])
    nc.compile()
    return nc


def _build_exp(loop_n=None):
    """Softmin-mode kernel.  Per pair of row tiles (A, B):
      - matmul both tiles' d2 chunks [128, 2048] into PSUM (full 8 banks)
      - probe: rowmin upper bound mhat over the first PROBE cols (DVE
        TT-min + tensor_reduce on PSUM), guarded to >= MGUARD; per-row
        scale = -CEXP/mhat
      - ACT: exp(scale_p * d2 + CEXP) -> bf16 scrap (discarded), fp32
        accum_out -> per-(tile, chunk) row sums: softmin on host
      - DVE: TT-min(psumA, psumB) -> fp16 pair mins merged into colacc
        (exact column mins; raw d2 survives the full dynamic range)
    Finale: fp16 PE transposes + min-reduce -> colmins.  Host: softmin
    rowmins from (rs, mh); exact colmins; sqrt + means."""
    import contextlib

    import concourse.bacc as bacc
    import concourse.tile as tile
    from concourse import mybir

    f16 = mybir.dt.float16
    bf16 = mybir.dt.bfloat16
    f32 = mybir.dt.float32
    MIN = mybir.AluOpType.min
    X = mybir.AxisListType.X
    EXPF = mybir.ActivationFunctionType.Exp

    nc = bacc.Bacc(
        "TRN2", target_bir_lowering=False, debug=False, num_devices=NCORES
    )
    stat = nc.dram_tensor("stat", [K, HALF], f16, kind="ExternalInput").ap()
    mov = nc.dram_tensor("mov", [K, N], f16, kind="ExternalInput").ap()
    ident = nc.dram_tensor("ident", [128, 128], f16, kind="ExternalInput").ap()
    cvec = nc.dram_tensor("cvec", [128, 1], f32, kind="ExternalInput").ap()
    rs_o = nc.dram_tensor("rs", [128, 4 * RT], f32, kind="ExternalOutput").ap()
    mh_o = nc.dram_tensor("mh", [128, RT], f32, kind="ExternalOutput").ap()
    colmin_o = nc.dram_tensor("colmin", [128, NT], f32, kind="ExternalOutput").ap()

    with tile.TileContext(nc) as tc:
        with (
            tc.tile_pool(name="persist", bufs=1) as persist,
            tc.tile_pool(name="psum", bufs=2, space="PSUM") as psum_pool,
            tc.tile_pool(name="escr", bufs=3) as escr_pool,
            tc.tile_pool(name="upool", bufs=2) as u_pool,
            tc.tile_pool(name="small", bufs=4) as small_pool,
        ):
            stat_sb = persist.tile([K, HALF], f16)
            mov_sb = persist.tile([K, N], f16)
            ident_sb = persist.tile([128, 128], f16)
            cvec_sb = persist.tile([128, 1], f32)
            colacc = persist.tile([128, N], f16)
            rs = persist.tile([128, 4 * RT], f32)
            mh = persist.tile([128, RT], f32)
            colmins = persist.tile([128, NT], f32)
            nc.sync.dma_start(stat_sb[:], stat)
            nc.sync.dma_start(mov_sb[:], mov)
            nc.sync.dma_start(ident_sb[:], ident)
            nc.sync.dma_start(cvec_sb[:], cvec)

            loop_cm = (
                tc.For_i(0, loop_n, 1)
                if loop_n is not None
                else contextlib.nullcontext()
            )
            with loop_cm:
                for t in range(RT // 2):
                    pair = (2 * t, 2 * t + 1)
                    pts = []
                    scales = []
                    for r_ in pair:
                        lhsT = stat_sb[:, r_ * 128 : (r_ + 1) * 128]
                        pt = psum_pool.tile([128, GW], f32, tag="pt")
                        for s in range(GW // MMW):
                            nc.tensor.matmul(
                                pt[:, s * MMW : (s + 1) * MMW],
                                lhsT,
                                mov_sb[:, s * MMW : (s + 1) * MMW],
                                start=True,
                                stop=True,
                            )
                        pts.append(pt)
                        # probe: rowmin upper bound over first PROBE cols
                        q = small_pool.tile([128, PROBE // 2], f16, tag="q")
                        nc.vector.tensor_tensor(
                            q[:], pt[:, : PROBE // 2], pt[:, PROBE // 2 : PROBE], MIN
                        )
                        tmp = small_pool.tile([128, 1], f32, tag="tmp")
                        nc.vector.tensor_reduce(tmp[:], q[:], X, MIN)
                        nc.vector.tensor_scalar_max(
                            mh[:, r_ : r_ + 1], tmp[:], MGUARD
                        )
                        inv = small_pool.tile([128, 1], f32, tag="inv")
                        nc.vector.reciprocal(inv[:], mh[:, r_ : r_ + 1])
                        sc = small_pool.tile([128, 1], f32, tag="sc")
                        nc.vector.tensor_scalar_mul(sc[:], inv[:], -CEXP)
                        scales.append(sc)

                    for g in range(4):
                        if g > 0:
                            pts = []
                            for r_ in pair:
                                lhsT = stat_sb[:, r_ * 128 : (r_ + 1) * 128]
                                pt = psum_pool.tile([128, GW], f32, tag="pt")
                                for s in range(GW // MMW):
                                    c0 = g * GW + s * MMW
                                    nc.tensor.matmul(
                                        pt[:, s * MMW : (s + 1) * MMW],
                                        lhsT,
                                        mov_sb[:, c0 : c0 + MMW],
                                        start=True,
                                        stop=True,
                                    )
                                pts.append(pt)
                        for pt, r_, sc in zip(pts, pair, scales):
                            e = escr_pool.tile([128, GW], bf16, tag="e")
                            nc.scalar.activation(
                                e[:],
                                pt[:],
                                EXPF,
                                bias=cvec_sb[:],
                                scale=sc[:],
                                accum_out=rs[:, 4 * r_ + g : 4 * r_ + g + 1],
                            )
                        sl = colacc[:, g * GW : (g + 1) * GW]
                        if t == 0:
                            nc.vector.tensor_tensor(sl, pts[0][:], pts[1][:], MIN)
                        else:
                            u = u_pool.tile([128, GW], f16, tag="u")
                            nc.vector.tensor_tensor(u[:], pts[0][:], pts[1][:], MIN)
                            nc.vector.tensor_tensor(sl, u[:], sl, MIN)

                # finale: fp16 transposes (8 blocks per PSUM tile via a
                # f16 view of the f32 pool tile) + packed min-reduce
                for j in range(NT // 8):
                    pf = psum_pool.tile([128, GW], f32, tag="pt")
                    pf16 = pf[:, : GW // 2].bitcast(f16)
                    for kk in range(8):
                        tb = 8 * j + kk
                        nc.tensor.matmul(
                            pf16[:, kk * 128 : (kk + 1) * 128],
                            colacc[:, tb * 128 : (tb + 1) * 128],
                            ident_sb[:],
                            is_transpose=True,
                            start=True,
                            stop=True,
                        )
                    nc.vector.tensor_reduce(
                        colmins[:, 8 * j : 8 * j + 8],
                        pf16[:, :1024].rearrange("p (b f) -> p b f", b=8),
                        X,
                        MIN,
                    )
            nc.sync.dma_start(rs_o, rs[:])
            nc.sync.dma_start(mh_o, mh[:])
            nc.sync.dma_start(colmin_o, colmins[:])
    nc.compile()
    return nc


def _build_timing(loop_n=None):
    """Mode-aware builder for the timing loop in test.py."""
    if MODE == "exp":
        return _build_exp(loop_n=loop_n)
    return _build_nc(loop_n=loop_n)


def _get_nc():
    if "nc" not in _CACHE:
        _CACHE["nc"] = _build_timing()
    return _CACHE["nc"]


def _split16(x):
    hi = x.astype(np.float16)
    lo = (x - hi.astype(np.float32)).astype(np.float16)
    return hi, lo


def _prep_batch(p, t):
    """p, t: [3, N] fp32 -> (S [K, N] fp16 stationary, M [K, N] fp16 moving)
    with d2[i, j] = sum_k S[k, i] * M[k, j] to ~1e-6 absolute."""
    p2 = (p * p).sum(axis=0)
    t2 = (t * t).sum(axis=0)
    S = np.empty((K, N), np.float16)
    M = np.empty((K, N), np.float16)
    S[0], S[1] = _split16(p2)
    M[0] = 1.0
    M[1] = 1.0
    S[2] = 1.0
    S[3] = 1.0
    M[2], M[3] = _split16(t2)
    for d in range(D):
        ah, al = _split16(-2.0 * p[d])
        bh, bl = _split16(t[d])
        base = 4 + 4 * d
        S[base + 0] = ah
        M[base + 0] = bh
        S[base + 1] = ah
        M[base + 1] = bl
        S[base + 2] = al
        M[base + 2] = bh
        S[base + 3] = al
        M[base + 3] = bl
    return S, M


def _make_in_maps(pred, target):
    pred = np.asarray(pred, dtype=np.float32)
    target = np.asarray(target, dtype=np.float32)
    in_maps = []
    for c in range(NCORES):
        b, h = divmod(c, 2)
        S, M = _prep_batch(pred[b], target[b])
        im = {
            "stat": np.ascontiguousarray(S[:, h * HALF : (h + 1) * HALF]),
            "mov": M,
        }
        if MODE == "exp":
            im["ident"] = np.eye(128, dtype=np.float16)
            im["cvec"] = np.full((128, 1), CEXP, np.float32)
        else:
            im["ident"] = np.eye(128, dtype=np.float32)
            im["mend"] = np.full((128, 1), float(N), np.float32)
        in_maps.append(im)
    return in_maps


def _finish_exp(results):
    """results per core: rs [128, 4*RT] f32 (chunk row sums, slot 4*r+g),
    mh [128, RT] f32 (guarded per-row scale), colmin [128, NT] f32."""
    row_total = 0.0
    col_total = 0.0
    for b in range(B):
        colparts = []
        for h in range(2):
            out = results[2 * b + h]
            rsum = (
                np.asarray(out["rs"], np.float64)
                .reshape(128, RT, 4)
                .sum(axis=2)
            )
            mhat = np.asarray(out["mh"], np.float64)
            rsum = np.maximum(rsum, 1e-300)
            rmin = mhat * (1.0 - np.log(rsum) / CEXP)
            rd2 = np.maximum(rmin.T.reshape(-1), 0.0)
            row_total += np.sqrt(rd2).sum()
            colparts.append(
                np.asarray(out["colmin"], dtype=np.float32).T.reshape(-1)
            )
        cd2 = np.maximum(np.minimum(colparts[0], colparts[1]), 0.0)
        col_total += np.sqrt(cd2, dtype=np.float64).sum()
    loss = row_total / (B * N) + col_total / (B * N)
    return np.array(loss, dtype=np.float32)


def _finish(results):
    """results: list of 8 dicts with 'rowmin' [128, RT] f32 and
    'colmin' [128, NT] f32 (colmin[p, t] = min_i d2[i, 128*t + p])."""
    row_total = 0.0
    col_total = 0.0
    for b in range(B):
        colparts = []
        for h in range(2):
            out = results[2 * b + h]
            rm = np.asarray(out["rowmin"], dtype=np.float32)  # [128, RT]
            # row index within half = r*128 + p -> transpose to [RT, 128]
            rd2 = np.maximum(rm.T.reshape(-1), 0.0)
            row_total += np.sqrt(rd2, dtype=np.float64).sum()
            # column j = 128*t + p -> transpose [NT, 128] then flatten
            colparts.append(
                np.asarray(out["colmin"], dtype=np.float32).T.reshape(-1)
            )
        cd2 = np.maximum(np.minimum(colparts[0], colparts[1]), 0.0)
        col_total += np.sqrt(cd2, dtype=np.float64).sum()
    loss = row_total / (B * N) + col_total / (B * N)
    return np.array(loss, dtype=np.float32)


def _run(in_maps, trace=False, nc=None):
    from concourse.bass_utils import run_bass_kernel_spmd

    if nc is None:
        nc = _get_nc()
    res = run_bass_kernel_spmd(
        nc, in_maps, list(range(NCORES)), trace=trace
    )
    return res


def kernel(pred, target):
    res = _run(_make_in_maps(pred, target), trace=False)
    if MODE == "exp":
        return _finish_exp(res.results)
    return _finish(res.results)



# revision 18
# speedup vs baseline: 1.4026x; 1.4026x over previous
"""Chamfer loss kernel for Trainium2 (8 NeuronCores).

Problem: pred/target [4, 3, 8192] channel-first point clouds.
loss = mean_i min_j ||p_i - t_j|| + mean_j min_i ||p_i - t_j||

d2[i,j] = ||p_i||^2 + ||t_j||^2 - 2 p_i.t_j is expressed as a single
K=16 fp16 matmul per tile (hi/lo splits keep |err| ~1e-6).  sqrt is
monotonic, so mins are taken over d2 and sqrt'd on host.

Sharding: core c -> (batch b = c//2, pred-row half h = c%2).  Each core
computes a [4096, 8192] block of d2 as 32 row tiles x 4 chunks of
[128, 2048] in PSUM and extracts
  - row mins  (min over the 8192 cols)  -> rowmin [128, 32]
  - col mins  (partial, per-partition)  -> colacc [128, 8192] -> PE
    transpose + reduce -> colmin [128, 64]
Host combines the tiny outputs.

The post-matmul reduction is the bottleneck.  ScalarE exports each PSUM
tile to SBUF fp16 (1 elem/cycle/lane); VectorE does both min directions
on the fp16 data at its 2x packed rate: one wide tensor_tensor min per
row tile into colacc, and a 4-level pairwise fold tree + tensor_reduce
for the row mins.  fp32 accumulation and the PE-transpose finale give
the cross-partition column mins.

Notes from exploration (this toolchain, axon/walrus):
  - nc.gpsimd.tensor_tensor/"Pool TensorTensor" fails walrus codegen
    (ISA engine check) - GPSIMD cannot help with elementwise min.
  - nc.vector.tensor_tensor_reduce compiles but crashes the device
    (NRT_EXEC_UNIT_UNRECOVERABLE) in every variant tried.
  - nc.vector.tensor_mask_reduce (InstTensorMaskReduce) also crashes
    the device the same way (rowmode="tmr").
  - nc.vector.tensor_tensor_scan(min, min) WORKS and is exact, but the
    recurrence runs at ~2 cycles/element - measured 441 us vs 316 us
    for the 2x fold tree (rowmode="scan").
  - nc.vector.pool_max runs at 1x (no packed mode) - slower than the
    2x fold tree.
  - DMA cannot read PSUM (dma_start asserts SBUF/DRAM source).
  - walrus enforces "only one non-scalar input may read PSUM" per DVE
    instruction (NCC_IBVF027): TT-min(psumA, psumB) is illegal, which
    kills cheap (0.5 cyc/elem) PSUM pair-merging for column mins.
  - TRN2 matmul output must be fp32 (16-bit PSUM is TRN3+), so DVE
    cannot read d2 from PSUM at its 2x packed rate.
  - ACT activation accum_out works (fp32-accurate chunk row sums, exp
    table rel err ~1.3e-3, per-partition scale/bias APs fine; ~187-280ns
    extra per accum read).  A softmin scheme (exp export with per-row
    probe-based scaling; validated numerically at rel err 2.5e-3 on this
    dataset) would free DVE of the rowmin fold, but without PSUM pair
    reads the column-min merge must read PSUM at 1x, and the scheme nets
    out slower than this kernel.  See MODE="exp" remnants.
  - fp8 export would disqualify DVE 2x packing (needs 2-byte dtypes).
  - Custom DVE ops (dve_ops Spec) run at 1x; only stock simple ops
    (TensorCopy/TensorScalar at 4x, TT/TMR at 2x) have fast modes, and
    tensor_scalar's accumulator is sum-only.
Hence all reduction work lands on DVE (~303 us/core modeled busy),
ACT ~263 us, PE ~116 us; modeled total 320 us, measured 290-325 us
(session noise is +/-15%).  Within this toolchain's constraint set
(2 reduction touches per element, DVE 2x max for min ops, ACT has no
min), this structure is at the floor: DVE = 1.0 cyc/elem (merge 0.5 +
fold 0.5) ~= 273 us busy + overheads, ACT export = 1.0 elem/cyc/lane
~= 218 us busy + overheads.

Each row tile is assigned a "way"; only "E" is usable here:
  E: ACT export fp16; DVE colmin TT + rowmin fold tree
  G/F/V (GPSIMD offload) and P (PSUM-direct DVE) are kept for
  reference but fail or lose on this toolchain.
"""

import numpy as np

B = 4
D = 3
N = 8192
HALF = N // 2  # pred rows per core
NCORES = 8
K = 16  # augmented contraction dim
RT = HALF // 128  # 32 row tiles per core
GW = 2048  # cols per PSUM tile (4 banks; 2 tiles in flight)
MMW = 512  # cols per matmul (one PSUM bank)
NT = N // 128  # 64 transpose blocks in the colmin finale

# Per-row-tile strategy, len 32.  r=0 must be E (its exports init colacc).
# GPSIMD (G/F/V) and tensor_tensor_reduce are rejected by this toolchain's
# walrus/runtime, so the default is all-E with the DVE fold tree.
WAYS_DEFAULT = "E" * 32

# "tree": baseline exact path (ACT fp16 export; DVE colacc TT + fold tree).
# "exp": softmin path — DEAD on this toolchain: the colmin pair-merge needs
#        TT(psumA, psumB) and walrus enforces "only one non-scalar input may
#        read PSUM" (NCC_IBVF027), leaving no cheap colmin source.
MODE = "tree"
CEXP = 80.0  # softmin sharpness: arg = CEXP * (1 - d2 / mhat_row)
PROBE = 512  # columns probed for the per-row upper bound mhat
MGUARD = 1e-4  # floor for mhat (negative-d2 / overflow guard)

_CACHE = {}


def _build_nc(ways=WAYS_DEFAULT, loop_n=None, rowmode="tree"):
    """loop_n: wrap the body in a device-side For_i loop executed loop_n
    times - constant program size, used for timing (delta between two
    loop_n values isolates pure HW execution time)."""
    import concourse.bacc as bacc
    import concourse.tile as tile
    from concourse import mybir

    assert len(ways) == RT and ways[0] == "E" and all(c in "EGFPV" for c in ways)
    f16 = mybir.dt.float16
    f32 = mybir.dt.float32
    MIN = mybir.AluOpType.min
    X = mybir.AxisListType.X
    BIG = 3.0e38

    uses_b = any(c in "GFV" for c in ways)

    nc = bacc.Bacc(
        "TRN2", target_bir_lowering=False, debug=False, num_devices=NCORES
    )
    stat = nc.dram_tensor("stat", [K, HALF], f16, kind="ExternalInput").ap()
    mov = nc.dram_tensor("mov", [K, N], f16, kind="ExternalInput").ap()
    ident = nc.dram_tensor("ident", [128, 128], f32, kind="ExternalInput").ap()
    mend = nc.dram_tensor("mend", [128, 1], f32, kind="ExternalInput").ap()
    rowmin_o = nc.dram_tensor("rowmin", [128, RT], f32, kind="ExternalOutput").ap()
    colmin_o = nc.dram_tensor("colmin", [128, NT], f32, kind="ExternalOutput").ap()

    with tile.TileContext(nc) as tc:
        with (
            tc.tile_pool(name="persist", bufs=1) as persist,
            tc.tile_pool(name="psum", bufs=2, space="PSUM") as psum_pool,
            tc.tile_pool(name="ckt", bufs=3) as ckt_pool,
            tc.tile_pool(name="scr", bufs=2) as scr_pool,
            tc.tile_pool(name="rp", bufs=2) as rp_pool,
        ):
            stat_sb = persist.tile([K, HALF], f16)
            mov_sb = persist.tile([K, N], f16)
            ident_sb = persist.tile([128, 128], f32)
            mend_sb = persist.tile([128, 1], f32)
            colacc = persist.tile([128, N], f16)
            colaccB = persist.tile([128, N], f32)
            rowmins = persist.tile([128, RT], f32)
            colmins = persist.tile([128, NT], f32)
            nc.sync.dma_start(stat_sb[:], stat)
            nc.sync.dma_start(mov_sb[:], mov)
            nc.sync.dma_start(ident_sb[:], ident)
            nc.sync.dma_start(mend_sb[:], mend)

            import contextlib

            loop_cm = (
                tc.For_i(0, loop_n, 1)
                if loop_n is not None
                else contextlib.nullcontext()
            )
            with loop_cm:
                b_inited = False
                for r, way in enumerate(ways):
                    lhsT = stat_sb[:, r * 128 : (r + 1) * 128]

                    if way == "P":
                        # PSUM-direct: both reductions read PSUM, no export.
                        rp = rp_pool.tile([128, 4], f32)
                        for g in range(4):
                            pt = psum_pool.tile([128, GW], f32, tag="pt")
                            for s in range(GW // MMW):
                                c0 = g * GW + s * MMW
                                nc.tensor.matmul(
                                    pt[:, s * MMW : (s + 1) * MMW],
                                    lhsT,
                                    mov_sb[:, c0 : c0 + MMW],
                                    start=True,
                                    stop=True,
                                )
                            csl = colacc[:, g * GW : (g + 1) * GW]
                            nc.vector.tensor_tensor(csl, pt[:], csl, MIN)
                            scr = scr_pool.tile([128, N // 2], f16)
                            nc.vector.tensor_tensor_reduce(
                                scr[:, : GW // 2],
                                pt[:, : GW // 2],
                                pt[:, GW // 2 :],
                                1.0,
                                BIG,
                                MIN,
                                MIN,
                                rp[:, g : g + 1],
                            )
                        nc.vector.tensor_reduce(
                            rowmins[:, r : r + 1], rp[:], X, MIN
                        )
                        continue

                    # Exported tiles.  First exported tile of each
                    # accumulator writes it directly (free init).
                    init_b = False
                    if way == "E" and r == 0:
                        dst = colacc
                    elif way in "GFV" and not b_inited:
                        dst = colaccB
                        b_inited = True
                        init_b = True
                    else:
                        dst = ckt_pool.tile([128, N], f16, tag="ck16")

                    for g in range(4):
                        pt = psum_pool.tile([128, GW], f32, tag="pt")
                        for s in range(GW // MMW):
                            c0 = g * GW + s * MMW
                            nc.tensor.matmul(
                                pt[:, s * MMW : (s + 1) * MMW],
                                lhsT,
                                mov_sb[:, c0 : c0 + MMW],
                                start=True,
                                stop=True,
                            )
                        dsl = dst[:, g * GW : (g + 1) * GW]
                        if way == "V":
                            nc.vector.tensor_copy(dsl, pt[:])
                        else:
                            nc.scalar.copy(dsl, pt[:])

                    # colmin merge
                    if way == "E":
                        if r > 0:
                            nc.vector.tensor_tensor(colacc[:], dst[:], colacc[:], MIN)
                    elif not init_b:  # G/F/V
                        nc.gpsimd.tensor_tensor(colaccB[:], dst[:], colaccB[:], MIN)

                    # rowmin
                    if way == "F":
                        nc.gpsimd.tensor_reduce(
                            rowmins[:, r : r + 1], dst[:], X, MIN
                        )
                    elif rowmode == "ttr":
                        scr = scr_pool.tile([128, N // 2], f16)
                        nc.vector.tensor_tensor_reduce(
                            scr[:],
                            dst[:, : N // 2],
                            dst[:, N // 2 :],
                            1.0,
                            BIG,
                            MIN,
                            MIN,
                            rowmins[:, r : r + 1],
                        )
                    elif rowmode == "ttr_bc":
                        # qr.py-style: dummy broadcast out, real accum
                        scr = scr_pool.tile([128, 1], f16, tag="scrbc")
                        nc.vector.tensor_tensor_reduce(
                            scr[:].broadcast_to((128, N // 2)),
                            dst[:, : N // 2],
                            dst[:, N // 2 :],
                            1.0,
                            BIG,
                            MIN,
                            MIN,
                            rowmins[:, r : r + 1],
                        )
                    elif rowmode == "ttr_add":
                        scr = scr_pool.tile([128, 1], f16, tag="scrbc")
                        nc.vector.tensor_tensor_reduce(
                            scr[:].broadcast_to((128, N // 2)),
                            dst[:, : N // 2],
                            dst[:, N // 2 :],
                            1.0,
                            0.0,
                            MIN,
                            mybir.AluOpType.add,
                            rowmins[:, r : r + 1],
                        )
                    elif rowmode == "ttr_rp":
                        scr = scr_pool.tile([128, N // 2], f16)
                        rp = rp_pool.tile([128, 4], f32)
                        nc.vector.tensor_tensor_reduce(
                            scr[:],
                            dst[:, : N // 2],
                            dst[:, N // 2 :],
                            1.0,
                            BIG,
                            MIN,
                            MIN,
                            rp[:, 0:1],
                        )
                        nc.vector.tensor_reduce(
                            rowmins[:, r : r + 1], rp[:, 0:1], X, MIN
                        )
                    elif rowmode == "pooltest":
                        # timing probe only: row-MAX via pool (wrong values)
                        nc.vector.pool_max(rowmins[:, r : r + 1], dst[:])
                    elif rowmode == "tmr":
                        # single 2x DVE instruction: full-range mask, min-accum
                        # (DEAD: InstTensorMaskReduce crashes the device, like
                        # InstTensorTensorReduce.)
                        scr = scr_pool.tile([128, N], f16, tag="tmrscr")
                        nc.vector.tensor_mask_reduce(
                            scr[:],
                            dst[:],
                            0.0,
                            mend_sb[:],
                            1.0,
                            BIG,
                            MIN,
                            accum_out=rowmins[:, r : r + 1],
                        )
                    elif rowmode == "scan":
                        # one 1x DVE scan over both tile halves:
                        #   state = min(state, dstL[t], dstR[t])
                        # final element = rowmin; extracted by ACT (slack).
                        scr = scr_pool.tile([128, N // 2], f16, tag="scanscr")
                        nc.vector.tensor_tensor_scan(
                            scr[:],
                            dst[:, : N // 2],
                            dst[:, N // 2 :],
                            BIG,
                            MIN,
                            MIN,
                        )
                        nc.scalar.copy(
                            rowmins[:, r : r + 1], scr[:, N // 2 - 1 : N // 2]
                        )
                    else:  # tree
                        scr = scr_pool.tile([128, N // 2], f16)
                        nc.vector.tensor_tensor(
                            scr[:], dst[:, : N // 2], dst[:, N // 2 :], MIN
                        )
                        scr2 = scr_pool.tile([128, N // 4], f16, tag="scr2")
                        nc.vector.tensor_tensor(
                            scr2[:], scr[:, : N // 4], scr[:, N // 4 :], MIN
                        )
                        scr3 = scr_pool.tile([128, N // 8], f16, tag="scr3")
                        nc.vector.tensor_tensor(
                            scr3[:], scr2[:, : N // 8], scr2[:, N // 8 :], MIN
                        )
                        scr4 = scr_pool.tile([128, N // 16], f16, tag="scr4")
                        nc.vector.tensor_tensor(
                            scr4[:], scr3[:, : N // 16], scr3[:, N // 16 :], MIN
                        )
                        scr5 = scr_pool.tile([128, N // 32], f16, tag="scr5")
                        nc.vector.tensor_tensor(
                            scr5[:], scr4[:, : N // 32], scr4[:, N // 32 :], MIN
                        )
                        nc.vector.tensor_reduce(
                            rowmins[:, r : r + 1], scr5[:], X, MIN
                        )

                # Fold the fp16 accumulator into the fp32 one; chunked so it
                # pipelines with the finale transposes.  The finale
                # transposes read fp32 (PE transpose out dtype must match).
                # The no-B cast-copy runs on ACT, which has slack.
                for q in range(4):
                    sl = slice(q * GW, (q + 1) * GW)
                    if uses_b:
                        nc.vector.tensor_tensor(
                            colaccB[:, sl], colacc[:, sl], colaccB[:, sl], MIN
                        )
                    else:
                        nc.scalar.copy(colaccB[:, sl], colacc[:, sl])

                # --- colmin finale: cross-partition reduce of colaccB ---
                # PE transpose of 128x128 blocks, packed min-reduce
                # 4 blocks per PSUM tile on DVE.
                for j in range(NT // 4):
                    pf = psum_pool.tile([128, GW], f32, tag="pt")
                    for kk in range(4):
                        t = 4 * j + kk
                        nc.tensor.matmul(
                            pf[:, kk * 128 : (kk + 1) * 128],
                            colaccB[:, t * 128 : (t + 1) * 128],
                            ident_sb[:],
                            is_transpose=True,
                            start=True,
                            stop=True,
                        )
                    nc.vector.tensor_reduce(
                        colmins[:, 4 * j : 4 * j + 4],
                        pf[:, :512].rearrange("p (b f) -> p b f", b=4),
                        X,
                        MIN,
                    )
            nc.sync.dma_start(rowmin_o, rowmins[:])
            nc.sync.dma_start(colmin_o, colmins[:])
    nc.compile()
    return nc


def _build_exp(loop_n=None):
    """Softmin-mode kernel.  Per pair of row tiles (A, B):
      - matmul both tiles' d2 chunks [128, 2048] into PSUM (full 8 banks)
      - probe: rowmin upper bound mhat over the first PROBE cols (DVE
        TT-min + tensor_reduce on PSUM), guarded to >= MGUARD; per-row
        scale = -CEXP/mhat
      - ACT: exp(scale_p * d2 + CEXP) -> bf16 scrap (discarded), fp32
        accum_out -> per-(tile, chunk) row sums: softmin on host
      - DVE: TT-min(psumA, psumB) -> fp16 pair mins merged into colacc
        (exact column mins; raw d2 survives the full dynamic range)
    Finale: fp16 PE transposes + min-reduce -> colmins.  Host: softmin
    rowmins from (rs, mh); exact colmins; sqrt + means."""
    import contextlib

    import concourse.bacc as bacc
    import concourse.tile as tile
    from concourse import mybir

    f16 = mybir.dt.float16
    bf16 = mybir.dt.bfloat16
    f32 = mybir.dt.float32
    MIN = mybir.AluOpType.min
    X = mybir.AxisListType.X
    EXPF = mybir.ActivationFunctionType.Exp

    nc = bacc.Bacc(
        "TRN2", target_bir_lowering=False, debug=False, num_devices=NCORES
    )
    stat = nc.dram_tensor("stat", [K, HALF], f16, kind="ExternalInput").ap()
    mov = nc.dram_tensor("mov", [K, N], f16, kind="ExternalInput").ap()
    ident = nc.dram_tensor("ident", [128, 128], f16, kind="ExternalInput").ap()
    cvec = nc.dram_tensor("cvec", [128, 1], f32, kind="ExternalInput").ap()
    rs_o = nc.dram_tensor("rs", [128, 4 * RT], f32, kind="ExternalOutput").ap()
    mh_o = nc.dram_tensor("mh", [128, RT], f32, kind="ExternalOutput").ap()
    colmin_o = nc.dram_tensor("colmin", [128, NT], f32, kind="ExternalOutput").ap()

    with tile.TileContext(nc) as tc:
        with (
            tc.tile_pool(name="persist", bufs=1) as persist,
            tc.tile_pool(name="psum", bufs=2, space="PSUM") as psum_pool,
            tc.tile_pool(name="escr", bufs=3) as escr_pool,
            tc.tile_pool(name="upool", bufs=2) as u_pool,
            tc.tile_pool(name="small", bufs=4) as small_pool,
        ):
            stat_sb = persist.tile([K, HALF], f16)
            mov_sb = persist.tile([K, N], f16)
            ident_sb = persist.tile([128, 128], f16)
            cvec_sb = persist.tile([128, 1], f32)
            colacc = persist.tile([128, N], f16)
            rs = persist.tile([128, 4 * RT], f32)
            mh = persist.tile([128, RT], f32)
            colmins = persist.tile([128, NT], f32)
            nc.sync.dma_start(stat_sb[:], stat)
            nc.sync.dma_start(mov_sb[:], mov)
            nc.sync.dma_start(ident_sb[:], ident)
            nc.sync.dma_start(cvec_sb[:], cvec)

            loop_cm = (
                tc.For_i(0, loop_n, 1)
                if loop_n is not None
                else contextlib.nullcontext()
            )
            with loop_cm:
                for t in range(RT // 2):
                    pair = (2 * t, 2 * t + 1)
                    pts = []
                    scales = []
                    for r_ in pair:
                        lhsT = stat_sb[:, r_ * 128 : (r_ + 1) * 128]
                        pt = psum_pool.tile([128, GW], f32, tag="pt")
                        for s in range(GW // MMW):
                            nc.tensor.matmul(
                                pt[:, s * MMW : (s + 1) * MMW],
                                lhsT,
                                mov_sb[:, s * MMW : (s + 1) * MMW],
                                start=True,
                                stop=True,
                            )
                        pts.append(pt)
                        # probe: rowmin upper bound over first PROBE cols
                        q = small_pool.tile([128, PROBE // 2], f16, tag="q")
                        nc.vector.tensor_tensor(
                            q[:], pt[:, : PROBE // 2], pt[:, PROBE // 2 : PROBE], MIN
                        )
                        tmp = small_pool.tile([128, 1], f32, tag="tmp")
                        nc.vector.tensor_reduce(tmp[:], q[:], X, MIN)
                        nc.vector.tensor_scalar_max(
                            mh[:, r_ : r_ + 1], tmp[:], MGUARD
                        )
                        inv = small_pool.tile([128, 1], f32, tag="inv")
                        nc.vector.reciprocal(inv[:], mh[:, r_ : r_ + 1])
                        sc = small_pool.tile([128, 1], f32, tag="sc")
                        nc.vector.tensor_scalar_mul(sc[:], inv[:], -CEXP)
                        scales.append(sc)

                    for g in range(4):
                        if g > 0:
                            pts = []
                            for r_ in pair:
                                lhsT = stat_sb[:, r_ * 128 : (r_ + 1) * 128]
                                pt = psum_pool.tile([128, GW], f32, tag="pt")
                                for s in range(GW // MMW):
                                    c0 = g * GW + s * MMW
                                    nc.tensor.matmul(
                                        pt[:, s * MMW : (s + 1) * MMW],
                                        lhsT,
                                        mov_sb[:, c0 : c0 + MMW],
                                        start=True,
                                        stop=True,
                                    )
                                pts.append(pt)
                        for pt, r_, sc in zip(pts, pair, scales):
                            e = escr_pool.tile([128, GW], bf16, tag="e")
                            nc.scalar.activation(
                                e[:],
                                pt[:],
                                EXPF,
                                bias=cvec_sb[:],
                                scale=sc[:],
                                accum_out=rs[:, 4 * r_ + g : 4 * r_ + g + 1],
                            )
                        sl = colacc[:, g * GW : (g + 1) * GW]
                        if t == 0:
                            nc.vector.tensor_tensor(sl, pts[0][:], pts[1][:], MIN)
                        else:
                            u = u_pool.tile([128, GW], f16, tag="u")
                            nc.vector.tensor_tensor(u[:], pts[0][:], pts[1][:], MIN)
                            nc.vector.tensor_tensor(sl, u[:], sl, MIN)

                # finale: fp16 transposes (8 blocks per PSUM tile via a
                # f16 view of the f32 pool tile) + packed min-reduce
                for j in range(NT // 8):
                    pf = psum_pool.tile([128, GW], f32, tag="pt")
                    pf16 = pf[:, : GW // 2].bitcast(f16)
                    for kk in range(8):
                        tb = 8 * j + kk
                        nc.tensor.matmul(
                            pf16[:, kk * 128 : (kk + 1) * 128],
                            colacc[:, tb * 128 : (tb + 1) * 128],
                            ident_sb[:],
                            is_transpose=True,
                            start=True,
                            stop=True,
                        )
                    nc.vector.tensor_reduce(
                        colmins[:, 8 * j : 8 * j + 8],
                        pf16[:, :1024].rearrange("p (b f) -> p b f", b=8),
                        X,
                        MIN,
                    )
            nc.sync.dma_start(rs_o, rs[:])
            nc.sync.dma_start(mh_o, mh[:])
            nc.sync.dma_start(colmin_o, colmins[:])
    nc.compile()
    return nc


def _build_timing(loop_n=None):
    """Mode-aware builder for the timing loop in test.py."""
    if MODE == "exp":
        return _build_exp(loop_n=loop_n)
    return _build_nc(loop_n=loop_n)


def _get_nc():
    if "nc" not in _CACHE:
        _CACHE["nc"] = _build_timing()
    return _CACHE["nc"]


def _split16(x):
    hi = x.astype(np.float16)
    lo = (x - hi.astype(np.float32)).astype(np.float16)
    return hi, lo


def _prep_batch(p, t):
    """p, t: [3, N] fp32 -> (S [K, N] fp16 stationary, M [K, N] fp16 moving)
    with d2[i, j] = sum_k S[k, i] * M[k, j] to ~1e-6 absolute."""
    p2 = (p * p).sum(axis=0)
    t2 = (t * t).sum(axis=0)
    S = np.empty((K, N), np.float16)
    M = np.empty((K, N), np.float16)
    S[0], S[1] = _split16(p2)
    M[0] = 1.0
    M[1] = 1.0
    S[2] = 1.0
    S[3] = 1.0
    M[2], M[3] = _split16(t2)
    for d in range(D):
        ah, al = _split16(-2.0 * p[d])
        bh, bl = _split16(t[d])
        base = 4 + 4 * d
        S[base + 0] = ah
        M[base + 0] = bh
        S[base + 1] = ah
        M[base + 1] = bl
        S[base + 2] = al
        M[base + 2] = bh
        S[base + 3] = al
        M[base + 3] = bl
    return S, M


def _make_in_maps(pred, target):
    pred = np.asarray(pred, dtype=np.float32)
    target = np.asarray(target, dtype=np.float32)
    in_maps = []
    for c in range(NCORES):
        b, h = divmod(c, 2)
        S, M = _prep_batch(pred[b], target[b])
        im = {
            "stat": np.ascontiguousarray(S[:, h * HALF : (h + 1) * HALF]),
            "mov": M,
        }
        if MODE == "exp":
            im["ident"] = np.eye(128, dtype=np.float16)
            im["cvec"] = np.full((128, 1), CEXP, np.float32)
        else:
            im["ident"] = np.eye(128, dtype=np.float32)
            im["mend"] = np.full((128, 1), float(N), np.float32)
        in_maps.append(im)
    return in_maps


def _finish_exp(results):
    """results per core: rs [128, 4*RT] f32 (chunk row sums, slot 4*r+g),
    mh [128, RT] f32 (guarded per-row scale), colmin [128, NT] f32."""
    row_total = 0.0
    col_total = 0.0
    for b in range(B):
        colparts = []
        for h in range(2):
            out = results[2 * b + h]
            rsum = (
                np.asarray(out["rs"], np.float64)
                .reshape(128, RT, 4)
                .sum(axis=2)
            )
            mhat = np.asarray(out["mh"], np.float64)
            rsum = np.maximum(rsum, 1e-300)
            rmin = mhat * (1.0 - np.log(rsum) / CEXP)
            rd2 = np.maximum(rmin.T.reshape(-1), 0.0)
            row_total += np.sqrt(rd2).sum()
            colparts.append(
                np.asarray(out["colmin"], dtype=np.float32).T.reshape(-1)
            )
        cd2 = np.maximum(np.minimum(colparts[0], colparts[1]), 0.0)
        col_total += np.sqrt(cd2, dtype=np.float64).sum()
    loss = row_total / (B * N) + col_total / (B * N)
    return np.array(loss, dtype=np.float32)


def _finish(results):
    """results: list of 8 dicts with 'rowmin' [128, RT] f32 and
    'colmin' [128, NT] f32 (colmin[p, t] = min_i d2[i, 128*t + p])."""
    row_total = 0.0
    col_total = 0.0
    for b in range(B):
        colparts = []
        for h in range(2):
            out = results[2 * b + h]
            rm = np.asarray(out["rowmin"], dtype=np.float32)  # [128, RT]
            # row index within half = r*128 + p -> transpose to [RT, 128]
            rd2 = np.maximum(rm.T.reshape(-1), 0.0)
            row_total += np.sqrt(rd2, dtype=np.float64).sum()
            # column j = 128*t + p -> transpose [NT, 128] then flatten
            colparts.append(
                np.asarray(out["colmin"], dtype=np.float32).T.reshape(-1)
            )
        cd2 = np.maximum(np.minimum(colparts[0], colparts[1]), 0.0)
        col_total += np.sqrt(cd2, dtype=np.float64).sum()
    loss = row_total / (B * N) + col_total / (B * N)
    return np.array(loss, dtype=np.float32)


def _run(in_maps, trace=False, nc=None):
    from concourse.bass_utils import run_bass_kernel_spmd

    if nc is None:
        nc = _get_nc()
    res = run_bass_kernel_spmd(
        nc, in_maps, list(range(NCORES)), trace=trace
    )
    return res


def kernel(pred, target):
    res = _run(_make_in_maps(pred, target), trace=False)
    if MODE == "exp":
        return _finish_exp(res.results)
    return _finish(res.results)



# revision 22
# speedup vs baseline: 1.4408x; 1.0273x over previous
"""Chamfer loss kernel for Trainium2 (8 NeuronCores).

Problem: pred/target [4, 3, 8192] channel-first point clouds.
loss = mean_i min_j ||p_i - t_j|| + mean_j min_i ||p_i - t_j||

d2[i,j] = ||p_i||^2 + ||t_j||^2 - 2 p_i.t_j is expressed as a single
K=16 fp16 matmul per tile (hi/lo splits keep |err| ~1e-6).  sqrt is
monotonic, so mins are taken over d2 and sqrt'd on host.

Sharding: core c -> (batch b = c//2, pred-row half h = c%2).  Each core
computes a [4096, 8192] block of d2 as 32 row tiles x 4 chunks of
[128, 2048] in PSUM and extracts
  - row mins  (min over the 8192 cols)  -> rowmin [128, 32]
  - col mins  (partial, per-partition)  -> colacc [128, 8192] -> PE
    transpose + reduce -> colmin [128, 64]
Host combines the tiny outputs.

The post-matmul reduction is the bottleneck.  ScalarE exports each PSUM
tile to SBUF fp16 (1 elem/cycle/lane); VectorE does both min directions
on the fp16 data at its 2x packed rate: one wide tensor_tensor min per
row tile into colacc, and a 4-level pairwise fold tree + tensor_reduce
for the row mins.  fp32 accumulation and the PE-transpose finale give
the cross-partition column mins.

Notes from exploration (this toolchain, axon/walrus):
  - nc.gpsimd.tensor_tensor/"Pool TensorTensor" fails walrus codegen
    (ISA engine check) - GPSIMD cannot help with elementwise min.
  - nc.vector.tensor_tensor_reduce compiles but crashes the device
    (NRT_EXEC_UNIT_UNRECOVERABLE) in every variant tried.
  - nc.vector.tensor_mask_reduce (InstTensorMaskReduce) also crashes
    the device the same way (rowmode="tmr").
  - nc.vector.tensor_tensor_scan(min, min) WORKS and is exact, but the
    recurrence runs at ~2 cycles/element - measured 441 us vs 316 us
    for the 2x fold tree (rowmode="scan").
  - nc.vector.pool_max runs at 1x (no packed mode) - slower than the
    2x fold tree.
  - DMA cannot read PSUM (dma_start asserts SBUF/DRAM source).
  - walrus enforces "only one non-scalar input may read PSUM" per DVE
    instruction (NCC_IBVF027): TT-min(psumA, psumB) is illegal, which
    kills cheap (0.5 cyc/elem) PSUM pair-merging for column mins.
  - TRN2 matmul output must be fp32 (16-bit PSUM is TRN3+), so DVE
    cannot read d2 from PSUM at its 2x packed rate.
  - ACT activation accum_out works (fp32-accurate chunk row sums, exp
    table rel err ~1.3e-3, per-partition scale/bias APs fine; ~187-280ns
    extra per accum read).  A softmin scheme (exp export with per-row
    probe-based scaling; validated numerically at rel err 2.5e-3 on this
    dataset) would free DVE of the rowmin fold, but without PSUM pair
    reads the column-min merge must read PSUM at 1x, and the scheme nets
    out slower than this kernel.  See MODE="exp" remnants.
  - fp8 export would disqualify DVE 2x packing (needs 2-byte dtypes).
  - Custom DVE ops (dve_ops Spec) run at 1x; only stock simple ops
    (TensorCopy/TensorScalar at 4x, TT/TMR at 2x) have fast modes, and
    tensor_scalar's accumulator is sum-only.
Hence all reduction work lands on DVE (~303 us/core modeled busy),
ACT ~263 us, PE ~116 us; modeled total 320 us, measured 290-325 us
(session noise is +/-15%).  Within this toolchain's constraint set
(2 reduction touches per element, DVE 2x max for min ops, ACT has no
min), this structure is at the floor: DVE = 1.0 cyc/elem (merge 0.5 +
fold 0.5) ~= 273 us busy + overheads, ACT export = 1.0 elem/cyc/lane
~= 218 us busy + overheads.

Each row tile is assigned a "way"; only "E" is usable here:
  E: ACT export fp16; DVE colmin TT + rowmin fold tree
  G/F/V (GPSIMD offload) and P (PSUM-direct DVE) are kept for
  reference but fail or lose on this toolchain.
"""

import numpy as np

B = 4
D = 3
N = 8192
HALF = N // 2  # pred rows per core
NCORES = 8
K = 16  # augmented contraction dim
RT = HALF // 128  # 32 row tiles per core
GW = 2048  # cols per PSUM tile (4 banks; 2 tiles in flight)
MMW = 512  # cols per matmul (one PSUM bank)
NT = N // 128  # 64 transpose blocks in the colmin finale

# Per-row-tile strategy, len 32.  r=0 must be E (its exports init colacc).
# GPSIMD (G/F/V) and tensor_tensor_reduce are rejected by this toolchain's
# walrus/runtime, so the default is all-E with the DVE fold tree.
WAYS_DEFAULT = "E" * 32

# "tree": baseline exact path (ACT fp16 export; DVE colacc TT + fold tree).
# "tree2": same math, instruction diet — pair-tile exports + 3D-AP fold
#        trees (half the DVE instructions) + fp16 finale (no ACT fold).
# "exp": softmin path — DEAD on this toolchain: the colmin pair-merge needs
#        TT(psumA, psumB) and walrus enforces "only one non-scalar input may
#        read PSUM" (NCC_IBVF027), leaving no cheap colmin source.
MODE = "tree2"
CEXP = 80.0  # softmin sharpness: arg = CEXP * (1 - d2 / mhat_row)
PROBE = 512  # columns probed for the per-row upper bound mhat
MGUARD = 1e-4  # floor for mhat (negative-d2 / overflow guard)

_CACHE = {}


def _build_nc(ways=WAYS_DEFAULT, loop_n=None, rowmode="tree"):
    """loop_n: wrap the body in a device-side For_i loop executed loop_n
    times - constant program size, used for timing (delta between two
    loop_n values isolates pure HW execution time)."""
    import concourse.bacc as bacc
    import concourse.tile as tile
    from concourse import mybir

    assert len(ways) == RT and ways[0] == "E" and all(c in "EGFPV" for c in ways)
    f16 = mybir.dt.float16
    f32 = mybir.dt.float32
    MIN = mybir.AluOpType.min
    X = mybir.AxisListType.X
    BIG = 3.0e38

    uses_b = any(c in "GFV" for c in ways)

    nc = bacc.Bacc(
        "TRN2", target_bir_lowering=False, debug=False, num_devices=NCORES
    )
    stat = nc.dram_tensor("stat", [K, HALF], f16, kind="ExternalInput").ap()
    mov = nc.dram_tensor("mov", [K, N], f16, kind="ExternalInput").ap()
    ident = nc.dram_tensor("ident", [128, 128], f32, kind="ExternalInput").ap()
    mend = nc.dram_tensor("mend", [128, 1], f32, kind="ExternalInput").ap()
    rowmin_o = nc.dram_tensor("rowmin", [128, RT], f32, kind="ExternalOutput").ap()
    colmin_o = nc.dram_tensor("colmin", [128, NT], f32, kind="ExternalOutput").ap()

    with tile.TileContext(nc) as tc:
        with (
            tc.tile_pool(name="persist", bufs=1) as persist,
            tc.tile_pool(name="psum", bufs=2, space="PSUM") as psum_pool,
            tc.tile_pool(name="ckt", bufs=3) as ckt_pool,
            tc.tile_pool(name="scr", bufs=2) as scr_pool,
            tc.tile_pool(name="rp", bufs=2) as rp_pool,
        ):
            stat_sb = persist.tile([K, HALF], f16)
            mov_sb = persist.tile([K, N], f16)
            ident_sb = persist.tile([128, 128], f32)
            mend_sb = persist.tile([128, 1], f32)
            colacc = persist.tile([128, N], f16)
            colaccB = persist.tile([128, N], f32)
            rowmins = persist.tile([128, RT], f32)
            colmins = persist.tile([128, NT], f32)
            nc.sync.dma_start(stat_sb[:], stat)
            nc.sync.dma_start(mov_sb[:], mov)
            nc.sync.dma_start(ident_sb[:], ident)
            nc.sync.dma_start(mend_sb[:], mend)

            import contextlib

            loop_cm = (
                tc.For_i(0, loop_n, 1)
                if loop_n is not None
                else contextlib.nullcontext()
            )
            with loop_cm:
                b_inited = False
                for r, way in enumerate(ways):
                    lhsT = stat_sb[:, r * 128 : (r + 1) * 128]

                    if way == "P":
                        # PSUM-direct: both reductions read PSUM, no export.
                        rp = rp_pool.tile([128, 4], f32)
                        for g in range(4):
                            pt = psum_pool.tile([128, GW], f32, tag="pt")
                            for s in range(GW // MMW):
                                c0 = g * GW + s * MMW
                                nc.tensor.matmul(
                                    pt[:, s * MMW : (s + 1) * MMW],
                                    lhsT,
                                    mov_sb[:, c0 : c0 + MMW],
                                    start=True,
                                    stop=True,
                                )
                            csl = colacc[:, g * GW : (g + 1) * GW]
                            nc.vector.tensor_tensor(csl, pt[:], csl, MIN)
                            scr = scr_pool.tile([128, N // 2], f16)
                            nc.vector.tensor_tensor_reduce(
                                scr[:, : GW // 2],
                                pt[:, : GW // 2],
                                pt[:, GW // 2 :],
                                1.0,
                                BIG,
                                MIN,
                                MIN,
                                rp[:, g : g + 1],
                            )
                        nc.vector.tensor_reduce(
                            rowmins[:, r : r + 1], rp[:], X, MIN
                        )
                        continue

                    # Exported tiles.  First exported tile of each
                    # accumulator writes it directly (free init).
                    init_b = False
                    if way == "E" and r == 0:
                        dst = colacc
                    elif way in "GFV" and not b_inited:
                        dst = colaccB
                        b_inited = True
                        init_b = True
                    else:
                        dst = ckt_pool.tile([128, N], f16, tag="ck16")

                    for g in range(4):
                        pt = psum_pool.tile([128, GW], f32, tag="pt")
                        for s in range(GW // MMW):
                            c0 = g * GW + s * MMW
                            nc.tensor.matmul(
                                pt[:, s * MMW : (s + 1) * MMW],
                                lhsT,
                                mov_sb[:, c0 : c0 + MMW],
                                start=True,
                                stop=True,
                            )
                        dsl = dst[:, g * GW : (g + 1) * GW]
                        if way == "V":
                            nc.vector.tensor_copy(dsl, pt[:])
                        else:
                            nc.scalar.copy(dsl, pt[:])

                    # colmin merge
                    if way == "E":
                        if r > 0:
                            nc.vector.tensor_tensor(colacc[:], dst[:], colacc[:], MIN)
                    elif not init_b:  # G/F/V
                        nc.gpsimd.tensor_tensor(colaccB[:], dst[:], colaccB[:], MIN)

                    # rowmin
                    if way == "F":
                        nc.gpsimd.tensor_reduce(
                            rowmins[:, r : r + 1], dst[:], X, MIN
                        )
                    elif rowmode == "ttr":
                        scr = scr_pool.tile([128, N // 2], f16)
                        nc.vector.tensor_tensor_reduce(
                            scr[:],
                            dst[:, : N // 2],
                            dst[:, N // 2 :],
                            1.0,
                            BIG,
                            MIN,
                            MIN,
                            rowmins[:, r : r + 1],
                        )
                    elif rowmode == "ttr_bc":
                        # qr.py-style: dummy broadcast out, real accum
                        scr = scr_pool.tile([128, 1], f16, tag="scrbc")
                        nc.vector.tensor_tensor_reduce(
                            scr[:].broadcast_to((128, N // 2)),
                            dst[:, : N // 2],
                            dst[:, N // 2 :],
                            1.0,
                            BIG,
                            MIN,
                            MIN,
                            rowmins[:, r : r + 1],
                        )
                    elif rowmode == "ttr_add":
                        scr = scr_pool.tile([128, 1], f16, tag="scrbc")
                        nc.vector.tensor_tensor_reduce(
                            scr[:].broadcast_to((128, N // 2)),
                            dst[:, : N // 2],
                            dst[:, N // 2 :],
                            1.0,
                            0.0,
                            MIN,
                            mybir.AluOpType.add,
                            rowmins[:, r : r + 1],
                        )
                    elif rowmode == "ttr_rp":
                        scr = scr_pool.tile([128, N // 2], f16)
                        rp = rp_pool.tile([128, 4], f32)
                        nc.vector.tensor_tensor_reduce(
                            scr[:],
                            dst[:, : N // 2],
                            dst[:, N // 2 :],
                            1.0,
                            BIG,
                            MIN,
                            MIN,
                            rp[:, 0:1],
                        )
                        nc.vector.tensor_reduce(
                            rowmins[:, r : r + 1], rp[:, 0:1], X, MIN
                        )
                    elif rowmode == "pooltest":
                        # timing probe only: row-MAX via pool (wrong values)
                        nc.vector.pool_max(rowmins[:, r : r + 1], dst[:])
                    elif rowmode == "tmr":
                        # single 2x DVE instruction: full-range mask, min-accum
                        # (DEAD: InstTensorMaskReduce crashes the device, like
                        # InstTensorTensorReduce.)
                        scr = scr_pool.tile([128, N], f16, tag="tmrscr")
                        nc.vector.tensor_mask_reduce(
                            scr[:],
                            dst[:],
                            0.0,
                            mend_sb[:],
                            1.0,
                            BIG,
                            MIN,
                            accum_out=rowmins[:, r : r + 1],
                        )
                    elif rowmode == "scan":
                        # one 1x DVE scan over both tile halves:
                        #   state = min(state, dstL[t], dstR[t])
                        # final element = rowmin; extracted by ACT (slack).
                        scr = scr_pool.tile([128, N // 2], f16, tag="scanscr")
                        nc.vector.tensor_tensor_scan(
                            scr[:],
                            dst[:, : N // 2],
                            dst[:, N // 2 :],
                            BIG,
                            MIN,
                            MIN,
                        )
                        nc.scalar.copy(
                            rowmins[:, r : r + 1], scr[:, N // 2 - 1 : N // 2]
                        )
                    else:  # tree
                        scr = scr_pool.tile([128, N // 2], f16)
                        nc.vector.tensor_tensor(
                            scr[:], dst[:, : N // 2], dst[:, N // 2 :], MIN
                        )
                        scr2 = scr_pool.tile([128, N // 4], f16, tag="scr2")
                        nc.vector.tensor_tensor(
                            scr2[:], scr[:, : N // 4], scr[:, N // 4 :], MIN
                        )
                        scr3 = scr_pool.tile([128, N // 8], f16, tag="scr3")
                        nc.vector.tensor_tensor(
                            scr3[:], scr2[:, : N // 8], scr2[:, N // 8 :], MIN
                        )
                        scr4 = scr_pool.tile([128, N // 16], f16, tag="scr4")
                        nc.vector.tensor_tensor(
                            scr4[:], scr3[:, : N // 16], scr3[:, N // 16 :], MIN
                        )
                        scr5 = scr_pool.tile([128, N // 32], f16, tag="scr5")
                        nc.vector.tensor_tensor(
                            scr5[:], scr4[:, : N // 32], scr4[:, N // 32 :], MIN
                        )
                        nc.vector.tensor_reduce(
                            rowmins[:, r : r + 1], scr5[:], X, MIN
                        )

                # Fold the fp16 accumulator into the fp32 one; chunked so it
                # pipelines with the finale transposes.  The finale
                # transposes read fp32 (PE transpose out dtype must match).
                # The no-B cast-copy runs on ACT, which has slack.
                for q in range(4):
                    sl = slice(q * GW, (q + 1) * GW)
                    if uses_b:
                        nc.vector.tensor_tensor(
                            colaccB[:, sl], colacc[:, sl], colaccB[:, sl], MIN
                        )
                    else:
                        nc.scalar.copy(colaccB[:, sl], colacc[:, sl])

                # --- colmin finale: cross-partition reduce of colaccB ---
                # PE transpose of 128x128 blocks, packed min-reduce
                # 4 blocks per PSUM tile on DVE.
                for j in range(NT // 4):
                    pf = psum_pool.tile([128, GW], f32, tag="pt")
                    for kk in range(4):
                        t = 4 * j + kk
                        nc.tensor.matmul(
                            pf[:, kk * 128 : (kk + 1) * 128],
                            colaccB[:, t * 128 : (t + 1) * 128],
                            ident_sb[:],
                            is_transpose=True,
                            start=True,
                            stop=True,
                        )
                    nc.vector.tensor_reduce(
                        colmins[:, 4 * j : 4 * j + 4],
                        pf[:, :512].rearrange("p (b f) -> p b f", b=4),
                        X,
                        MIN,
                    )
            nc.sync.dma_start(rowmin_o, rowmins[:])
            nc.sync.dma_start(colmin_o, colmins[:])
    nc.compile()
    return nc


def _build_tree2(loop_n=None):
    """Instruction-diet variant of the exact tree kernel:
      - tiles exported in PAIRS into one [128, 2*N] fp16 buffer, so each
        fold-tree level is ONE 3D-AP TT covering both tiles (5 levels x 16
        pairs instead of 10 x 16), and the final tensor_reduce handles both
        tiles' rowmins at once
      - fp16 finale: PE transposes the fp16 colacc directly into an f16
        bitcast view of the PSUM tile (no fp32 colaccB fold pass on ACT,
        shorter loop-boundary serial chain), 16 blocks per min-reduce
    Same math as rowmode="tree"; DVE instruction count per tile drops
    ~224 -> ~128 and the ACT fold (4 copies) disappears."""
    import contextlib

    import concourse.bacc as bacc
    import concourse.tile as tile
    from concourse import mybir

    f16 = mybir.dt.float16
    f32 = mybir.dt.float32
    MIN = mybir.AluOpType.min
    X = mybir.AxisListType.X

    nc = bacc.Bacc(
        "TRN2", target_bir_lowering=False, debug=False, num_devices=NCORES
    )
    stat = nc.dram_tensor("stat", [K, HALF], f16, kind="ExternalInput").ap()
    mov = nc.dram_tensor("mov", [K, N], f16, kind="ExternalInput").ap()
    ident = nc.dram_tensor("ident", [128, 128], f16, kind="ExternalInput").ap()
    rowmin_o = nc.dram_tensor("rowmin", [128, RT], f32, kind="ExternalOutput").ap()
    colmin_o = nc.dram_tensor("colmin", [128, NT], f32, kind="ExternalOutput").ap()

    with tile.TileContext(nc) as tc:
        with (
            tc.tile_pool(name="persist", bufs=1) as persist,
            tc.tile_pool(name="psum", bufs=2, space="PSUM") as psum_pool,
            tc.tile_pool(name="ckt", bufs=2) as ckt_pool,
            tc.tile_pool(name="scr", bufs=2) as scr_pool,
        ):
            stat_sb = persist.tile([K, HALF], f16)
            mov_sb = persist.tile([K, N], f16)
            ident_sb = persist.tile([128, 128], f16)
            colacc = persist.tile([128, N], f16)
            rowmins = persist.tile([128, RT], f32)
            colmins = persist.tile([128, NT], f32)
            nc.sync.dma_start(stat_sb[:], stat)
            nc.sync.dma_start(mov_sb[:], mov)
            nc.sync.dma_start(ident_sb[:], ident)

            loop_cm = (
                tc.For_i(0, loop_n, 1)
                if loop_n is not None
                else contextlib.nullcontext()
            )
            with loop_cm:
                for t in range(RT // 2):
                    pair = ckt_pool.tile([128, 2 * N], f16, tag="pair")
                    for half, r_ in enumerate((2 * t, 2 * t + 1)):
                        lhsT = stat_sb[:, r_ * 128 : (r_ + 1) * 128]
                        base = half * N
                        for g in range(4):
                            pt = psum_pool.tile([128, GW], f32, tag="pt")
                            for s in range(GW // MMW):
                                c0 = g * GW + s * MMW
                                nc.tensor.matmul(
                                    pt[:, s * MMW : (s + 1) * MMW],
                                    lhsT,
                                    mov_sb[:, c0 : c0 + MMW],
                                    start=True,
                                    stop=True,
                                )
                            nc.scalar.copy(
                                pair[:, base + g * GW : base + (g + 1) * GW],
                                pt[:],
                            )

                    # colmin merge (per tile; pair 0 initialises colacc)
                    if t == 0:
                        nc.vector.tensor_tensor(
                            colacc[:], pair[:, :N], pair[:, N:], MIN
                        )
                    else:
                        nc.vector.tensor_tensor(
                            colacc[:], pair[:, :N], colacc[:], MIN
                        )
                        nc.vector.tensor_tensor(
                            colacc[:], pair[:, N:], colacc[:], MIN
                        )

                    # rowmin fold tree, both tiles per instruction via 3D APs
                    pv = pair[:].rearrange("p (b f) -> p b f", b=2)
                    s1 = scr_pool.tile([128, N], f16, tag="s1")
                    v1 = s1[:].rearrange("p (b f) -> p b f", b=2)
                    nc.vector.tensor_tensor(
                        v1, pv[:, :, : N // 2], pv[:, :, N // 2 :], MIN
                    )
                    s2 = scr_pool.tile([128, N // 2], f16, tag="s2")
                    v2 = s2[:].rearrange("p (b f) -> p b f", b=2)
                    nc.vector.tensor_tensor(
                        v2, v1[:, :, : N // 4], v1[:, :, N // 4 :], MIN
                    )
                    s3 = scr_pool.tile([128, N // 4], f16, tag="s3")
                    v3 = s3[:].rearrange("p (b f) -> p b f", b=2)
                    nc.vector.tensor_tensor(
                        v3, v2[:, :, : N // 8], v2[:, :, N // 8 :], MIN
                    )
                    s4 = scr_pool.tile([128, N // 8], f16, tag="s4")
                    v4 = s4[:].rearrange("p (b f) -> p b f", b=2)
                    nc.vector.tensor_tensor(
                        v4, v3[:, :, : N // 16], v3[:, :, N // 16 :], MIN
                    )
                    s5 = scr_pool.tile([128, N // 16], f16, tag="s5")
                    v5 = s5[:].rearrange("p (b f) -> p b f", b=2)
                    nc.vector.tensor_tensor(
                        v5, v4[:, :, : N // 32], v4[:, :, N // 32 :], MIN
                    )
                    nc.vector.tensor_reduce(
                        rowmins[:, 2 * t : 2 * t + 2], v5, X, MIN
                    )

                # fp16 finale: transpose colacc blocks into an f16 view of
                # the f32 PSUM tile, then packed min-reduce (16 blocks/round)
                for j in range(NT // 16):
                    pf = psum_pool.tile([128, GW], f32, tag="pt")
                    pf16 = pf[:].bitcast(f16)
                    for kk in range(16):
                        tb = 16 * j + kk
                        nc.tensor.matmul(
                            pf16[:, kk * 128 : (kk + 1) * 128],
                            colacc[:, tb * 128 : (tb + 1) * 128],
                            ident_sb[:],
                            is_transpose=True,
                            start=True,
                            stop=True,
                        )
                    nc.vector.tensor_reduce(
                        colmins[:, 16 * j : 16 * j + 16],
                        pf16[:, :2048].rearrange("p (b f) -> p b f", b=16),
                        X,
                        MIN,
                    )
            nc.sync.dma_start(rowmin_o, rowmins[:])
            nc.sync.dma_start(colmin_o, colmins[:])
    nc.compile()
    return nc


def _build_exp(loop_n=None):
    """Softmin-mode kernel.  Per pair of row tiles (A, B):
      - matmul both tiles' d2 chunks [128, 2048] into PSUM (full 8 banks)
      - probe: rowmin upper bound mhat over the first PROBE cols (DVE
        TT-min + tensor_reduce on PSUM), guarded to >= MGUARD; per-row
        scale = -CEXP/mhat
      - ACT: exp(scale_p * d2 + CEXP) -> bf16 scrap (discarded), fp32
        accum_out -> per-(tile, chunk) row sums: softmin on host
      - DVE: TT-min(psumA, psumB) -> fp16 pair mins merged into colacc
        (exact column mins; raw d2 survives the full dynamic range)
    Finale: fp16 PE transposes + min-reduce -> colmins.  Host: softmin
    rowmins from (rs, mh); exact colmins; sqrt + means."""
    import contextlib

    import concourse.bacc as bacc
    import concourse.tile as tile
    from concourse import mybir

    f16 = mybir.dt.float16
    bf16 = mybir.dt.bfloat16
    f32 = mybir.dt.float32
    MIN = mybir.AluOpType.min
    X = mybir.AxisListType.X
    EXPF = mybir.ActivationFunctionType.Exp

    nc = bacc.Bacc(
        "TRN2", target_bir_lowering=False, debug=False, num_devices=NCORES
    )
    stat = nc.dram_tensor("stat", [K, HALF], f16, kind="ExternalInput").ap()
    mov = nc.dram_tensor("mov", [K, N], f16, kind="ExternalInput").ap()
    ident = nc.dram_tensor("ident", [128, 128], f16, kind="ExternalInput").ap()
    cvec = nc.dram_tensor("cvec", [128, 1], f32, kind="ExternalInput").ap()
    rs_o = nc.dram_tensor("rs", [128, 4 * RT], f32, kind="ExternalOutput").ap()
    mh_o = nc.dram_tensor("mh", [128, RT], f32, kind="ExternalOutput").ap()
    colmin_o = nc.dram_tensor("colmin", [128, NT], f32, kind="ExternalOutput").ap()

    with tile.TileContext(nc) as tc:
        with (
            tc.tile_pool(name="persist", bufs=1) as persist,
            tc.tile_pool(name="psum", bufs=2, space="PSUM") as psum_pool,
            tc.tile_pool(name="escr", bufs=3) as escr_pool,
            tc.tile_pool(name="upool", bufs=2) as u_pool,
            tc.tile_pool(name="small", bufs=4) as small_pool,
        ):
            stat_sb = persist.tile([K, HALF], f16)
            mov_sb = persist.tile([K, N], f16)
            ident_sb = persist.tile([128, 128], f16)
            cvec_sb = persist.tile([128, 1], f32)
            colacc = persist.tile([128, N], f16)
            rs = persist.tile([128, 4 * RT], f32)
            mh = persist.tile([128, RT], f32)
            colmins = persist.tile([128, NT], f32)
            nc.sync.dma_start(stat_sb[:], stat)
            nc.sync.dma_start(mov_sb[:], mov)
            nc.sync.dma_start(ident_sb[:], ident)
            nc.sync.dma_start(cvec_sb[:], cvec)

            loop_cm = (
                tc.For_i(0, loop_n, 1)
                if loop_n is not None
                else contextlib.nullcontext()
            )
            with loop_cm:
                for t in range(RT // 2):
                    pair = (2 * t, 2 * t + 1)
                    pts = []
                    scales = []
                    for r_ in pair:
                        lhsT = stat_sb[:, r_ * 128 : (r_ + 1) * 128]
                        pt = psum_pool.tile([128, GW], f32, tag="pt")
                        for s in range(GW // MMW):
                            nc.tensor.matmul(
                                pt[:, s * MMW : (s + 1) * MMW],
                                lhsT,
                                mov_sb[:, s * MMW : (s + 1) * MMW],
                                start=True,
                                stop=True,
                            )
                        pts.append(pt)
                        # probe: rowmin upper bound over first PROBE cols
                        q = small_pool.tile([128, PROBE // 2], f16, tag="q")
                        nc.vector.tensor_tensor(
                            q[:], pt[:, : PROBE // 2], pt[:, PROBE // 2 : PROBE], MIN
                        )
                        tmp = small_pool.tile([128, 1], f32, tag="tmp")
                        nc.vector.tensor_reduce(tmp[:], q[:], X, MIN)
                        nc.vector.tensor_scalar_max(
                            mh[:, r_ : r_ + 1], tmp[:], MGUARD
                        )
                        inv = small_pool.tile([128, 1], f32, tag="inv")
                        nc.vector.reciprocal(inv[:], mh[:, r_ : r_ + 1])
                        sc = small_pool.tile([128, 1], f32, tag="sc")
                        nc.vector.tensor_scalar_mul(sc[:], inv[:], -CEXP)
                        scales.append(sc)

                    for g in range(4):
                        if g > 0:
                            pts = []
                            for r_ in pair:
                                lhsT = stat_sb[:, r_ * 128 : (r_ + 1) * 128]
                                pt = psum_pool.tile([128, GW], f32, tag="pt")
                                for s in range(GW // MMW):
                                    c0 = g * GW + s * MMW
                                    nc.tensor.matmul(
                                        pt[:, s * MMW : (s + 1) * MMW],
                                        lhsT,
                                        mov_sb[:, c0 : c0 + MMW],
                                        start=True,
                                        stop=True,
                                    )
                                pts.append(pt)
                        for pt, r_, sc in zip(pts, pair, scales):
                            e = escr_pool.tile([128, GW], bf16, tag="e")
                            nc.scalar.activation(
                                e[:],
                                pt[:],
                                EXPF,
                                bias=cvec_sb[:],
                                scale=sc[:],
                                accum_out=rs[:, 4 * r_ + g : 4 * r_ + g + 1],
                            )
                        sl = colacc[:, g * GW : (g + 1) * GW]
                        if t == 0:
                            nc.vector.tensor_tensor(sl, pts[0][:], pts[1][:], MIN)
                        else:
                            u = u_pool.tile([128, GW], f16, tag="u")
                            nc.vector.tensor_tensor(u[:], pts[0][:], pts[1][:], MIN)
                            nc.vector.tensor_tensor(sl, u[:], sl, MIN)

                # finale: fp16 transposes (8 blocks per PSUM tile via a
                # f16 view of the f32 pool tile) + packed min-reduce
                for j in range(NT // 8):
                    pf = psum_pool.tile([128, GW], f32, tag="pt")
                    pf16 = pf[:, : GW // 2].bitcast(f16)
                    for kk in range(8):
                        tb = 8 * j + kk
                        nc.tensor.matmul(
                            pf16[:, kk * 128 : (kk + 1) * 128],
                            colacc[:, tb * 128 : (tb + 1) * 128],
                            ident_sb[:],
                            is_transpose=True,
                            start=True,
                            stop=True,
                        )
                    nc.vector.tensor_reduce(
                        colmins[:, 8 * j : 8 * j + 8],
                        pf16[:, :1024].rearrange("p (b f) -> p b f", b=8),
                        X,
                        MIN,
                    )
            nc.sync.dma_start(rs_o, rs[:])
            nc.sync.dma_start(mh_o, mh[:])
            nc.sync.dma_start(colmin_o, colmins[:])
    nc.compile()
    return nc


def _build_timing(loop_n=None):
    """Mode-aware builder for the timing loop in test.py."""
    if MODE == "exp":
        return _build_exp(loop_n=loop_n)
    if MODE == "tree2":
        return _build_tree2(loop_n=loop_n)
    return _build_nc(loop_n=loop_n)


def _get_nc():
    if "nc" not in _CACHE:
        _CACHE["nc"] = _build_timing()
    return _CACHE["nc"]


def _split16(x):
    hi = x.astype(np.float16)
    lo = (x - hi.astype(np.float32)).astype(np.float16)
    return hi, lo


def _prep_batch(p, t):
    """p, t: [3, N] fp32 -> (S [K, N] fp16 stationary, M [K, N] fp16 moving)
    with d2[i, j] = sum_k S[k, i] * M[k, j] to ~1e-6 absolute."""
    p2 = (p * p).sum(axis=0)
    t2 = (t * t).sum(axis=0)
    S = np.empty((K, N), np.float16)
    M = np.empty((K, N), np.float16)
    S[0], S[1] = _split16(p2)
    M[0] = 1.0
    M[1] = 1.0
    S[2] = 1.0
    S[3] = 1.0
    M[2], M[3] = _split16(t2)
    for d in range(D):
        ah, al = _split16(-2.0 * p[d])
        bh, bl = _split16(t[d])
        base = 4 + 4 * d
        S[base + 0] = ah
        M[base + 0] = bh
        S[base + 1] = ah
        M[base + 1] = bl
        S[base + 2] = al
        M[base + 2] = bh
        S[base + 3] = al
        M[base + 3] = bl
    return S, M


def _make_in_maps(pred, target):
    pred = np.asarray(pred, dtype=np.float32)
    target = np.asarray(target, dtype=np.float32)
    in_maps = []
    for c in range(NCORES):
        b, h = divmod(c, 2)
        S, M = _prep_batch(pred[b], target[b])
        im = {
            "stat": np.ascontiguousarray(S[:, h * HALF : (h + 1) * HALF]),
            "mov": M,
        }
        if MODE == "exp":
            im["ident"] = np.eye(128, dtype=np.float16)
            im["cvec"] = np.full((128, 1), CEXP, np.float32)
        elif MODE == "tree2":
            im["ident"] = np.eye(128, dtype=np.float16)
        else:
            im["ident"] = np.eye(128, dtype=np.float32)
            im["mend"] = np.full((128, 1), float(N), np.float32)
        in_maps.append(im)
    return in_maps


def _finish_exp(results):
    """results per core: rs [128, 4*RT] f32 (chunk row sums, slot 4*r+g),
    mh [128, RT] f32 (guarded per-row scale), colmin [128, NT] f32."""
    row_total = 0.0
    col_total = 0.0
    for b in range(B):
        colparts = []
        for h in range(2):
            out = results[2 * b + h]
            rsum = (
                np.asarray(out["rs"], np.float64)
                .reshape(128, RT, 4)
                .sum(axis=2)
            )
            mhat = np.asarray(out["mh"], np.float64)
            rsum = np.maximum(rsum, 1e-300)
            rmin = mhat * (1.0 - np.log(rsum) / CEXP)
            rd2 = np.maximum(rmin.T.reshape(-1), 0.0)
            row_total += np.sqrt(rd2).sum()
            colparts.append(
                np.asarray(out["colmin"], dtype=np.float32).T.reshape(-1)
            )
        cd2 = np.maximum(np.minimum(colparts[0], colparts[1]), 0.0)
        col_total += np.sqrt(cd2, dtype=np.float64).sum()
    loss = row_total / (B * N) + col_total / (B * N)
    return np.array(loss, dtype=np.float32)


def _finish(results):
    """results: list of 8 dicts with 'rowmin' [128, RT] f32 and
    'colmin' [128, NT] f32 (colmin[p, t] = min_i d2[i, 128*t + p])."""
    row_total = 0.0
    col_total = 0.0
    for b in range(B):
        colparts = []
        for h in range(2):
            out = results[2 * b + h]
            rm = np.asarray(out["rowmin"], dtype=np.float32)  # [128, RT]
            # row index within half = r*128 + p -> transpose to [RT, 128]
            rd2 = np.maximum(rm.T.reshape(-1), 0.0)
            row_total += np.sqrt(rd2, dtype=np.float64).sum()
            # column j = 128*t + p -> transpose [NT, 128] then flatten
            colparts.append(
                np.asarray(out["colmin"], dtype=np.float32).T.reshape(-1)
            )
        cd2 = np.maximum(np.minimum(colparts[0], colparts[1]), 0.0)
        col_total += np.sqrt(cd2, dtype=np.float64).sum()
    loss = row_total / (B * N) + col_total / (B * N)
    return np.array(loss, dtype=np.float32)


def _run(in_maps, trace=False, nc=None):
    from concourse.bass_utils import run_bass_kernel_spmd

    if nc is None:
        nc = _get_nc()
    res = run_bass_kernel_spmd(
        nc, in_maps, list(range(NCORES)), trace=trace
    )
    return res


def kernel(pred, target):
    res = _run(_make_in_maps(pred, target), trace=False)
    if MODE == "exp":
        return _finish_exp(res.results)
    return _finish(res.results)

